# revision 31
# baseline (speedup 1.0000x reference)
"""Trainium2 Bass kernel for nn_BaseLineModel (hierarchical sentence->doc model).

v4 design (fp8 DoubleRow + xbar transposes + interleaved LSTM):
  - position-major groups: group g = sentence-slot g of the core's 4 docs.
  - embedding gather (indirect DMA, fp32) -> ACT cast bf16 (E padded to 384)
    -> DMA xbar transpose (SBUF->SBUF, off the PE) -> DVE cast fp8 ebt/tail.
  - conv as fp8 DoubleRow matmuls: 5 full-E (256-row) pairs per tap
    + 3 tail pairs covering E rows 256..299 (shift-staged tile) with the
    conv bias folded in via a constant-ones row.
  - token attention: exp applied directly to the logits PSUM (tanh
    dropped: |y| << 1; ba0 dropped: softmax shift-invariant); pad columns
    zeroed so weighted sums run as contiguous segmented tensor_reduce.
  - LSTM interleaved one step per group behind the conv pipeline; whh/x
    projections as DoubleRow (K=256 in one MM); sigmoid via tanh(x/2)
    (keeps ACT in the exp/tanh table set); h stored as H=2h in fp8 with
    weight scales folded on host.
  - sentence attention: same exp-direct softmax; sigmoid output via tanh.

Data-parallel over docs: core k handles docs 4k..4k+3; host concatenates
the 8 [1,4] outputs. No collectives.
"""
import sys

for _p in ("/opt/trn_rl_repo", "/root/.axon_site/_ro/trn_rl_repo"):
    if _p not in sys.path:
        sys.path.insert(0, _p)

from contextlib import ExitStack

import numpy as np
import ml_dtypes

import concourse.bass as bass
from concourse import mybir
from concourse.bass import IndirectOffsetOnAxis
from concourse.bass_utils import run_bass_kernel_spmd
from concourse.tile import TileContext

from concourse.vector_clock import ScopedClock


class _TC(TileContext):
    """TileContext that limits every instruction to a single sem wait
    (this walrus rejects multiple sync waits on one instruction); extra
    waits are spilled onto preceding same-engine nops."""

    def _commit_instruction(self, inst, lazy_reg_writes: bool = True):
        si = getattr(inst, "sync_info", None)
        if (
            si is not None
            and si.on_wait
            and len(si.on_wait) > 1
            and inst.engine != mybir.EngineType.Unassigned
        ):
            waits = list(si.on_wait)
            inst.sync_info = mybir.SyncInfo(
                on_wait=[waits[-1]], on_update=list(si.on_update or []))
            eng = self.nc.engines[inst.engine]
            for w in waits[:-1]:
                nop = eng.nop().ins
                nop.sync_info = mybir.SyncInfo(on_wait=[w], on_update=[])
        return super()._commit_instruction(inst, lazy_reg_writes)

    def _drain_and_barrier(self, tick_clock, wait_clock):
        carrier = self.nc.sync.nop().ins
        wait_clock.add_sem_waits(
            carrier, ScopedClock({None: tick_clock.global_clock}))
        si = carrier.sync_info
        if si is not None and si.on_wait and len(si.on_wait) > 1:
            waits = list(si.on_wait)
            carrier.sync_info = mybir.SyncInfo(
                on_wait=[waits[0]], on_update=list(si.on_update or []))
            for w in waits[1:]:
                n2 = self.nc.sync.nop().ins
                n2.sync_info = mybir.SyncInfo(on_wait=[w], on_update=[])
        self.nc.sync.drain()
        self.nc.all_engine_barrier()
        assert self.sems is not None
        popped = self.nc._tile_sem_poison_stack.pop()
        assert popped is self._sem_poison
        self.nc.clear_and_free_semaphores(list(self.sems.allocated().values()))
        self.nc.all_engine_barrier()


BF16 = mybir.dt.bfloat16
FP32 = mybir.dt.float32
FP8 = mybir.dt.float8e4
INT32 = mybir.dt.int32
AF = mybir.ActivationFunctionType
OP = mybir.AluOpType
AX = mybir.AxisListType
DR = mybir.MatmulPerfMode.DoubleRow

B, S, L = 32, 64, 128
PER = 32
TOTAL = B * PER
V, E, F, W, H = 30000, 300, 256, 5, 256
T = L - W + 1            # 124 valid conv positions
NCORES = 8
DPC = B // NCORES        # 4 docs per core
bf16 = ml_dtypes.bfloat16
f8 = ml_dtypes.float8_e4m3

LSTM_LAG = 4             # lstm step t issues during group t+LSTM_LAG
XP_CHUNK = 4             # x-projection batched every 4 slots


def build_nc(S_eff: int, debug_taps=False):
    assert S_eff % XP_CHUNK == 0
    NLOC = S_eff * DPC
    S4 = S_eff * DPC

    nc = bass.Bass()
    dbg = {}
    if debug_taps:
        dbg["c0"] = nc.dram_tensor("dbg_c0", [128, 2, 528], FP8,
                                   kind="ExternalOutput")
        dbg["s0"] = nc.dram_tensor("dbg_s0", [128, 2, S_eff, DPC], FP8,
                                   kind="ExternalOutput")
        dbg["xpt"] = nc.dram_tensor("dbg_xpt", [128, 8, S_eff, DPC], FP32,
                                    kind="ExternalOutput")
        dbg["hsb"] = nc.dram_tensor("dbg_hsb", [128, 2, S_eff, DPC], FP8,
                                    kind="ExternalOutput")
        dbg["ebt"] = nc.dram_tensor("dbg_ebt", [128, 2, 528], FP8,
                                    kind="ExternalOutput")
        dbg["tail"] = nc.dram_tensor("dbg_tail", [45, 3, 528], FP8,
                                     kind="ExternalOutput")

    # ---- DRAM I/O ----
    d_idx = nc.dram_tensor("idx_t", [128, NLOC], INT32, kind="ExternalInput")
    d_wemb = nc.dram_tensor("wemb", [V, E], FP32, kind="ExternalInput")
    d_wdr = nc.dram_tensor("wdr", [128, 2, W, 2, 128], FP8, kind="ExternalInput")
    d_wtail = nc.dram_tensor("wtail", [45, 2, 3, 2, 128], FP8,
                             kind="ExternalInput")
    d_wa0 = nc.dram_tensor("wa0dr", [128, 2, 2, 128], FP8, kind="ExternalInput")
    d_wih = nc.dram_tensor("wih", [128, 2, 8, 128], FP8, kind="ExternalInput")
    d_bx = nc.dram_tensor("bihhh_x", [128, 8, XP_CHUNK, DPC], FP32,
                          kind="ExternalInput")
    d_whh = nc.dram_tensor("whh", [128, 2, 8, 128], FP8, kind="ExternalInput")
    d_wa1 = nc.dram_tensor("wa1dr", [128, 2, 2, 128], FP8, kind="ExternalInput")
    d_wo = nc.dram_tensor("wo_t", [128, 2], BF16, kind="ExternalInput")
    d_boh = nc.dram_tensor("bo_half", [1, 1], FP32, kind="ExternalInput")
    d_out = nc.dram_tensor("out", [1, DPC], FP32, kind="ExternalOutput")

    with _TC(nc) as tc, ExitStack() as ctx:
        consts = ctx.enter_context(tc.tile_pool(name="consts", bufs=1))

        idx_sb = consts.tile([128, NLOC], INT32)
        nc.sync.dma_start(out=idx_sb[:, :], in_=d_idx[:, :])
        wdr_sb = consts.tile([128, 2, W, 2, 128], FP8)
        nc.sync.dma_start(out=wdr_sb[:, :, :, :, :], in_=d_wdr[:, :, :, :, :])
        wtail_sb = consts.tile([45, 2, 3, 2, 128], FP8)
        nc.sync.dma_start(out=wtail_sb[:, :, :, :, :],
                          in_=d_wtail[:, :, :, :, :])
        wa0_sb = consts.tile([128, 2, 2, 128], FP8)
        nc.sync.dma_start(out=wa0_sb[:, :, :, :], in_=d_wa0[:, :, :, :])
        wih_sb = consts.tile([128, 2, 8, 128], FP8)
        nc.sync.dma_start(out=wih_sb[:, :, :, :], in_=d_wih[:, :, :, :])
        bx_sb = consts.tile([128, 8, XP_CHUNK, DPC], FP32)
        nc.sync.dma_start(out=bx_sb[:, :, :, :], in_=d_bx[:, :, :, :])
        whh_sb = consts.tile([128, 2, 8, 128], FP8)
        nc.sync.dma_start(out=whh_sb[:, :, :, :], in_=d_whh[:, :, :, :])
        wa1_sb = consts.tile([128, 2, 2, 128], FP8)
        nc.sync.dma_start(out=wa1_sb[:, :, :, :], in_=d_wa1[:, :, :, :])
        wo_sb = consts.tile([128, 2], BF16)
        nc.sync.dma_start(out=wo_sb[:, :], in_=d_wo[:, :])
        boh_sb = consts.tile([1, 1], FP32)
        nc.sync.dma_start(out=boh_sb[:, :], in_=d_boh[:, :])

        # persistent state
        s0T = consts.tile([128, 2, S_eff, DPC], FP8)
        xpt = consts.tile([128, 8, S_eff, DPC], FP32)
        hsb = consts.tile([128, 2, S_eff, DPC], FP8)
        cC = consts.tile([128, 2, DPC], FP32)          # C = 2*c state
        nc.vector.memset(cC[:, :, :], 0.0)

        ident32 = consts.tile([128, 128], FP32)
        from concourse.masks import make_identity
        make_identity(nc, ident32[:, :])
        identf8 = consts.tile([128, 128], FP8)
        nc.vector.tensor_copy(out=identf8[:, :], in_=ident32[:, :])
        # fp8 gathered-embedding staging (manually rotated, 3 deep)
        emb8_0 = consts.tile([128, DPC, E], FP8)
        emb8_1 = consts.tile([128, DPC, E], FP8)
        emb8_2 = consts.tile([128, DPC, E], FP8)
        emb8s = [emb8_0, emb8_1, emb8_2]
        # tail fp8: slot 0 = ebT[c-4], slot 1 = ebT[c-3] (+ row 44 ones for
        # the conv bias), slot 2 = zeros (DR filler half for the w4 tap)
        tail_0 = consts.tile([45, 3, 528], FP8)
        tail_1 = consts.tile([45, 3, 528], FP8)
        tail_2 = consts.tile([45, 3, 528], FP8)
        tails = [tail_0, tail_1, tail_2]
        for tl in tails:
            nc.vector.memset(tl[0:45, 2, :], 0.0)
            nc.vector.memset(tl[32:45, 1, :], 1.0)

        with (
            tc.tile_pool(name="emb", bufs=5) as p_emb,
            tc.tile_pool(name="ebt8", bufs=3) as p_ebt8,
            tc.tile_pool(name="c0", bufs=2) as p_c0,
            tc.tile_pool(name="ex", bufs=2) as p_ex,
            tc.tile_pool(name="small", bufs=3) as p_small,
            tc.tile_pool(name="lst", bufs=3) as p_lst,
            tc.tile_pool(name="mm", bufs=2, space="PSUM") as p_mm,
            tc.tile_pool(name="tg", bufs=2, space="PSUM") as p_tg,
            tc.tile_pool(name="tp", bufs=1, space="PSUM") as p_tp,
        ):
            embs = {}
            tps = {}
            ebt8s = {}

            def gather(g):
                emb = p_emb.tile([128, DPC, E], FP32, tag="emb")
                for d in range(DPC):
                    nc.gpsimd.indirect_dma_start(
                        out=emb[:, d, :],
                        out_offset=None,
                        in_=d_wemb[:, :],
                        in_offset=IndirectOffsetOnAxis(
                            ap=idx_sb[:, DPC * g + d:DPC * g + d + 1], axis=0),
                    )
                embs[g] = emb

            def cast1(g):
                # fp32 -> fp8 on the ACT engine
                nc.scalar.activation(out=emb8s[g % 3][:, :, :],
                                     in_=embs.pop(g)[:, :, :], func=AF.Copy)

            def transp(g):
                # PE transpose (fp8 matmul vs identity) into PSUM
                src = emb8s[g % 3]
                tpm = p_tp.tile([128, 2, 512], FP32, tag="tpm")
                tpt = p_tg.tile([128, 512], FP32, tag="tg")
                for s in range(DPC):
                    for ec in range(2):
                        nc.tensor.matmul(
                            out=tpm[:, ec, 128 * s:128 * (s + 1)],
                            lhsT=src[:, s, 128 * ec:128 * (ec + 1)],
                            rhs=identf8[:, :], start=True, stop=True)
                    nc.tensor.matmul(
                        out=tpt[0:44, 128 * s:128 * (s + 1)],
                        lhsT=src[:, s, 256:300],
                        rhs=identf8[:, :], start=True, stop=True)
                tps[g] = (tpm, tpt)

            def cast2_fixed(g):
                # PSUM fp32 -> fp8 staging for the DoubleRow conv
                tpm, tpt = tps.pop(g)
                ebt = p_ebt8.tile([128, 2, 528], FP8, tag="ebt")
                nc.scalar.activation(out=ebt[:, :, 0:512], in_=tpm[:, :, :],
                                     func=AF.Copy)
                tail = tails[g % 3]
                nc.vector.tensor_copy(out=tail[0:44, 0, 4:516],
                                      in_=tpt[0:44, 0:512])
                nc.vector.tensor_copy(out=tail[0:44, 1, 4:515],
                                      in_=tpt[0:44, 1:512])
                ebt8s[g] = (ebt, tail)

            pending_bias = []

            def xp_mms(j0):
                xps = p_tg.tile([128, 512], FP32, tag="tg")
                xv = xps[:, :].rearrange("p (g x) -> p g x", g=8)
                for gt in range(8):
                    nc.tensor.matmul(
                        out=xv[:, gt, 0:XP_CHUNK * DPC],
                        lhsT=wih_sb[:, 0:2, gt, :],
                        rhs=s0T[:, 0:2, j0:j0 + XP_CHUNK, :],
                        start=True, stop=True, perf_mode=DR,
                    )
                pending_bias.append((j0, xps))

            def xp_bias_flush():
                while pending_bias:
                    j0, xps = pending_bias.pop()
                    xv = xps[:, :].rearrange("p (g x) -> p g x", g=8)
                    nc.vector.tensor_tensor(
                        out=xpt[:, :, j0:j0 + XP_CHUNK, :],
                        in0=xv[:, :, 0:XP_CHUNK * DPC].rearrange(
                            "p g (j d) -> p g j d", d=DPC),
                        in1=bx_sb[:, :, :, :], op=OP.add)

            def lstm_step(t):
                if t == 0:
                    ga_ap = xpt[:, :, 0, :]
                else:
                    gps = p_tg.tile([128, 512], FP32, tag="tg")
                    gv = gps[:, :].rearrange("p (g x) -> p g x", g=8)
                    for gt in range(8):
                        nc.tensor.matmul(
                            out=gv[:, gt, 0:DPC],
                            lhsT=whh_sb[:, 0:2, gt, :],
                            rhs=hsb[:, 0:2, t - 1, :],
                            start=True, stop=True, perf_mode=DR,
                        )
                    ga = p_lst.tile([128, 8, DPC], FP32, tag="ga")
                    nc.vector.tensor_tensor(
                        out=ga[:, :, :], in0=gv[:, :, 0:DPC],
                        in1=xpt[:, :, t, :], op=OP.add)
                    ga_ap = ga[:, :, :]
                tga = p_lst.tile([128, 8, DPC], FP32, tag="tga")
                nc.scalar.activation(out=tga[:, :, :], in_=ga_ap,
                                     func=AF.Tanh, scale=0.5)
                a = p_lst.tile([128, 2, DPC], FP32, tag="a")
                b = p_lst.tile([128, 2, DPC], FP32, tag="b")
                # a = (t_f + 1) * C = 4 f c ; b = (t_i + 1) * g~ = 2 i g~
                nc.vector.scalar_tensor_tensor(
                    out=a[:, :, :], in0=tga[:, 2:4, :], scalar=1.0,
                    in1=cC[:, :, :], op0=OP.add, op1=OP.mult)
                nc.vector.scalar_tensor_tensor(
                    out=b[:, :, :], in0=tga[:, 0:2, :], scalar=1.0,
                    in1=tga[:, 4:6, :], op0=OP.add, op1=OP.mult)
                # C = a/2 + b  (= 2 c_new)
                nc.vector.scalar_tensor_tensor(
                    out=cC[:, :, :], in0=a[:, :, :], scalar=0.5,
                    in1=b[:, :, :], op0=OP.mult, op1=OP.add)
                tch = p_lst.tile([128, 2, DPC], FP32, tag="tch")
                nc.scalar.activation(out=tch[:, :, :], in_=cC[:, :, :],
                                     func=AF.Tanh, scale=0.5)
                # H = (t_o + 1) * tanh(c) = 2 h   (fp8)
                nc.vector.scalar_tensor_tensor(
                    out=hsb[:, :, t, :], in0=tga[:, 6:8, :], scalar=1.0,
                    in1=tch[:, :, :], op0=OP.add, op1=OP.mult)

            # ---- warmup ----
            for i in range(min(4, S_eff)):
                gather(i)
            for i in range(min(3, S_eff)):
                cast1(i)
            transp(0)
            cast2_fixed(0)
            if S_eff > 1:
                transp(1)

            for g in range(S_eff):
                if g + 4 < S_eff:
                    gather(g + 4)
                if g + 3 < S_eff:
                    cast1(g + 3)
                if g + 1 < S_eff:
                    cast2_fixed(g + 1)
                xp_bias_flush()
                if g - LSTM_LAG >= 0:
                    lstm_step(g - LSTM_LAG)

                ebt, tail = ebt8s.pop(g)

                # ---- conv: fp8 DoubleRow accumulation ----
                cps = p_mm.tile([128, 2, 512], FP32, tag="mm")
                for fc in range(2):
                    for w in range(W):
                        nc.tensor.matmul(
                            out=cps[:, fc, 0:508],
                            lhsT=wdr_sb[:, :, w, fc, :],
                            rhs=ebt[:, 0:2, w:w + 508],
                            start=(w == 0), stop=False, perf_mode=DR)
                    nc.tensor.matmul(
                        out=cps[:, fc, 0:508],
                        lhsT=wtail_sb[0:44, :, 0, fc, :],
                        rhs=tail[0:44, 0:2, 4:512],
                        start=False, stop=False, perf_mode=DR)
                    nc.tensor.matmul(
                        out=cps[:, fc, 0:508],
                        lhsT=wtail_sb[0:44, :, 1, fc, :],
                        rhs=tail[0:44, 0:2, 6:514],
                        start=False, stop=False, perf_mode=DR)
                    nc.tensor.matmul(
                        out=cps[:, fc, 0:508],
                        lhsT=wtail_sb[0:45, :, 2, fc, :],
                        rhs=tail[0:45, 1:3, 7:515],
                        start=False, stop=True, perf_mode=DR)

                # transposes for group g+2 (PE), behind the conv in the queue
                if g + 2 < S_eff:
                    transp(g + 2)

                c0 = p_c0.tile([128, 2, 528], FP8, tag="c0")
                nc.scalar.activation(out=c0[:, :, 0:512], in_=cps[:, :, :],
                                     func=AF.Tanh)
                if debug_taps and g == 0:
                    nc.sync.dma_start(out=dbg["c0"][:, :, :], in_=c0[:, :, :])
                    nc.sync.dma_start(out=dbg["ebt"][:, :, :], in_=ebt[:, :, :])
                    nc.sync.dma_start(out=dbg["tail"][:, :, :],
                                      in_=tail[:, :, :])

                # ---- token attention ----
                lps = p_mm.tile([128, 2, 512], FP32, tag="mm")
                for mc in range(2):
                    nc.tensor.matmul(
                        out=lps[:, mc, 0:508],
                        lhsT=wa0_sb[:, :, mc, :],
                        rhs=c0[:, 0:2, 0:508],
                        start=True, stop=True, perf_mode=DR)
                ex = p_ex.tile([128, 2, 512], BF16, tag="ex")
                nc.scalar.activation(out=ex[:, :, 0:512], in_=lps[:, :, :],
                                     func=AF.Exp)
                # zero the 4 pad columns after each sentence so the segmented
                # reductions below can run over contiguous 128-wide blocks
                nc.vector.memset(
                    ex[:, :, :].rearrange(
                        "p m (s t) -> p m s t", t=128)[:, :, :, T:128], 0.0)
                prod = p_ex.tile([128, 2, 512], BF16, tag="prod")
                nc.vector.tensor_tensor(
                    out=prod[:, :, :], in0=ex[:, :, :],
                    in1=c0[:, :, 0:512], op=OP.mult)
                num = p_small.tile([128, 2, DPC], BF16, tag="num")
                den = p_small.tile([128, 2, DPC], BF16, tag="den")
                with nc.allow_low_precision("softmax sums of ~124 bounded "
                                            "terms; fp8 path dominates error"):
                    nc.vector.tensor_reduce(
                        out=num[:, :, :],
                        in_=prod[:, :, :].rearrange(
                            "p m (s t) -> p m s t", t=128),
                        axis=AX.X, op=OP.add)
                    nc.vector.tensor_reduce(
                        out=den[:, :, :],
                        in_=ex[:, :, :].rearrange(
                            "p m (s t) -> p m s t", t=128),
                        axis=AX.X, op=OP.add)
                rden = p_small.tile([128, 2, DPC], FP32, tag="rden")
                nc.vector.reciprocal(out=rden[:, :, :], in_=den[:, :, :])
                nc.vector.tensor_tensor(
                    out=s0T[:, :, g, :], in0=num[:, :, :], in1=rden[:, :, :],
                    op=OP.mult)

                # ---- x-projection for the previous XP_CHUNK slots ----
                if g % XP_CHUNK == 0 and g >= XP_CHUNK:
                    xp_mms(g - XP_CHUNK)

            # ---- drain: last xp chunk + remaining LSTM steps ----
            xp_mms(S_eff - XP_CHUNK)
            xp_bias_flush()
            for t in range(max(0, S_eff - LSTM_LAG), S_eff):
                lstm_step(t)

            if debug_taps:
                nc.sync.dma_start(out=dbg["s0"][:, :, :, :],
                                  in_=s0T[:, :, :, :])
                nc.sync.dma_start(out=dbg["xpt"][:, :, :, :],
                                  in_=xpt[:, :, :, :])
                nc.sync.dma_start(out=dbg["hsb"][:, :, :, :],
                                  in_=hsb[:, :, :, :])

            # ---- sentence attention + output ----
            l1ps = p_mm.tile([128, 2, 512], FP32, tag="mm")
            for mc in range(2):
                nc.tensor.matmul(
                    out=l1ps[:, mc, 0:S4],
                    lhsT=wa1_sb[:, :, mc, :],
                    rhs=hsb[:, 0:2, :, :],
                    start=True, stop=True, perf_mode=DR)
            ex1 = p_ex.tile([128, 2, S4], BF16, tag="ex1")
            nc.scalar.activation(out=ex1[:, :, :], in_=l1ps[:, :, 0:S4],
                                 func=AF.Exp)
            prod1 = p_ex.tile([128, 2, S_eff, DPC], BF16, tag="prod1")
            nc.vector.tensor_tensor(
                out=prod1[:, :, :, :],
                in0=ex1[:, :, :].rearrange("p m (t d) -> p m t d", d=DPC),
                in1=hsb[:, :, :, :], op=OP.mult)
            num1 = p_small.tile([128, 2, DPC], FP32, tag="num1")
            den1 = p_small.tile([128, 2, DPC], FP32, tag="den1")
            nc.vector.tensor_reduce(
                out=num1[:, :, :],
                in_=prod1[:, :, :, :].rearrange("p m t d -> p m d t"),
                axis=AX.X, op=OP.add)
            nc.vector.tensor_reduce(
                out=den1[:, :, :],
                in_=ex1[:, :, :].rearrange(
                    "p m (t d) -> p m d t", d=DPC),
                axis=AX.X, op=OP.add)
            nc.vector.reciprocal(out=den1[:, :, :], in_=den1[:, :, :])
            s1 = p_small.tile([128, 2, DPC], BF16, tag="s1")
            nc.vector.tensor_tensor(
                out=s1[:, :, :], in0=num1[:, :, :], in1=den1[:, :, :],
                op=OP.mult)
            ops = p_tg.tile([128, 512], FP32, tag="tg")
            for kc in range(2):
                nc.tensor.matmul(
                    out=ops[0:1, 0:DPC],
                    lhsT=wo_sb[:, kc:kc + 1],
                    rhs=s1[:, kc, :],
                    start=(kc == 0), stop=(kc == 1),
                )
            y = p_small.tile([1, DPC], FP32, tag="y")
            nc.scalar.activation(
                out=y[:, :], in_=ops[0:1, 0:DPC],
                func=AF.Tanh, bias=boh_sb[0:1, 0:1], scale=0.5)
            nc.vector.tensor_scalar(
                out=y[:, :], in0=y[:, :],
                scalar1=0.5, scalar2=0.5, op0=OP.mult, op1=OP.add)
            nc.sync.dma_start(out=d_out[:, :], in_=y[:, :])

    return nc


def _host_prep(inputs):
    inp = {k: np.asarray(v) for k, v in inputs.items()}
    tok = inp["input"].astype(np.int32)
    num_sent = inp["num_sent"].astype(np.int64)

    assert np.all(num_sent == num_sent[0]), "non-uniform num_sent unsupported"
    S_eff = int(num_sent[0])
    assert S_eff % XP_CHUNK == 0 and S_eff >= XP_CHUNK

    wc = np.asarray(inp["Wconv"], np.float32)     # [F, 1, W, E]
    bconv = np.asarray(inp["bconv"], np.float32)  # [F]

    wdr = np.zeros((128, 2, W, 2, 128), f8)
    for w in range(W):
        for fc in range(2):
            for k2 in range(2):
                blk = wc[128 * fc:128 * (fc + 1), 0, w,
                         128 * k2:128 * (k2 + 1)]      # [m, p]
                wdr[:, k2, w, fc, :] = blk.T.astype(f8)
    wt = np.zeros((45, 2, 3, 2, 128), np.float32)      # [p,k2,pair,fc,m]
    for fc in range(2):
        msl = slice(128 * fc, 128 * (fc + 1))
        wt[0:44, 0, 0, fc, :] = wc[msl, 0, 0, 256:300].T   # w0
        wt[0:44, 1, 0, fc, :] = wc[msl, 0, 1, 256:300].T   # w1
        wt[0:44, 0, 1, fc, :] = wc[msl, 0, 2, 256:300].T   # w2
        wt[0:44, 1, 1, fc, :] = wc[msl, 0, 3, 256:300].T   # w3
        wt[0:44, 0, 2, fc, :] = wc[msl, 0, 4, 256:300].T   # w4
        wt[44, 0, 2, fc, :] = bconv[msl]                   # bias via ones row

    wa0 = np.asarray(inp["Wa0"], np.float32)
    wa0dr = np.zeros((128, 2, 2, 128), f8)
    for k2 in range(2):
        for mc in range(2):
            wa0dr[:, k2, mc, :] = wa0[128 * k2:128 * (k2 + 1),
                                      128 * mc:128 * (mc + 1)].astype(f8)

    # gate order (i0,i1,f0,f1,g0,g1,o0,o1); g-gate rows x2 (tanh via scale .5)
    wih = np.asarray(inp["Wih"], np.float32)      # [4H, F]
    whh = np.asarray(inp["Whh"], np.float32)      # [4H, H]
    bih = np.asarray(inp["bih"], np.float32) + np.asarray(inp["bhh"], np.float32)
    gscale = np.ones((4 * H, 1), np.float32)
    gscale[2 * H:3 * H] = 2.0
    wih_eff = wih * gscale
    whh_eff = (whh * 0.5) * gscale                 # h stored as H=2h
    bx1 = (bih * gscale[:, 0])                     # [4H]
    wih_t = np.zeros((128, 2, 8, 128), f8)
    whh_t = np.zeros((128, 2, 8, 128), f8)
    for kc in range(2):
        for gt in range(8):
            wih_t[:, kc, gt, :] = wih_eff[128 * gt:128 * (gt + 1),
                                          128 * kc:128 * (kc + 1)].T.astype(f8)
            whh_t[:, kc, gt, :] = whh_eff[128 * gt:128 * (gt + 1),
                                          128 * kc:128 * (kc + 1)].T.astype(f8)
    bx = np.zeros((128, 8, XP_CHUNK, DPC), np.float32)
    for gt in range(8):
        bx[:, gt, :, :] = bx1[128 * gt:128 * (gt + 1)][:, None, None]

    wa1 = np.asarray(inp["Wa1"], np.float32) * 0.5  # h = H/2
    wa1dr = np.zeros((128, 2, 2, 128), f8)
    for k2 in range(2):
        for mc in range(2):
            wa1dr[:, k2, mc, :] = wa1[128 * k2:128 * (k2 + 1),
                                      128 * mc:128 * (mc + 1)].astype(f8)

    wo = np.asarray(inp["Wo"], np.float32) * 0.5    # s1 arrives as 2*s1
    wo_t = wo[:, 0].reshape(2, 128).T.astype(bf16).copy()
    boh = (0.5 * np.asarray(inp["bo"], np.float32)).reshape(1, 1)

    wemb = np.ascontiguousarray(inp["Wemb"], np.float32)

    in_maps = []
    for k in range(NCORES):
        idx_t = np.zeros((128, S_eff * DPC), np.int32)
        for g in range(S_eff):
            for d in range(DPC):
                sent = (k * DPC + d) * PER + g
                idx_t[:, DPC * g + d] = tok[sent]
        in_maps.append({
            "idx_t": idx_t, "wemb": wemb,
            "wdr": np.ascontiguousarray(wdr),
            "wtail": np.ascontiguousarray(wt.astype(f8)),
            "wa0dr": wa0dr, "wih": wih_t, "bihhh_x": bx,
            "whh": whh_t, "wa1dr": wa1dr, "wo_t": wo_t, "bo_half": boh,
        })
    return S_eff, in_maps


_NC_CACHE = {}


def kernel(**inputs) -> np.ndarray:
    S_eff, in_maps = _host_prep(inputs)
    if S_eff not in _NC_CACHE:
        _NC_CACHE[S_eff] = build_nc(S_eff)
    nc = _NC_CACHE[S_eff]
    res = run_bass_kernel_spmd(nc, in_maps, core_ids=list(range(NCORES)))
    out = np.zeros((B, 1), np.float32)
    for k in range(NCORES):
        out[k * DPC:(k + 1) * DPC, 0] = res.results[k]["out"][0]
    return out


# revision 33
# speedup vs baseline: 1.1611x; 1.1611x over previous
"""Trainium2 Bass kernel for nn_BaseLineModel (hierarchical sentence->doc model).

v4 design (fp8 DoubleRow + xbar transposes + interleaved LSTM):
  - position-major groups: group g = sentence-slot g of the core's 4 docs.
  - embedding gather (indirect DMA, fp32) -> ACT cast bf16 (E padded to 384)
    -> DMA xbar transpose (SBUF->SBUF, off the PE) -> DVE cast fp8 ebt/tail.
  - conv as fp8 DoubleRow matmuls: 5 full-E (256-row) pairs per tap
    + 3 tail pairs covering E rows 256..299 (shift-staged tile) with the
    conv bias folded in via a constant-ones row.
  - token attention: exp applied directly to the logits PSUM (tanh
    dropped: |y| << 1; ba0 dropped: softmax shift-invariant); pad columns
    zeroed so weighted sums run as contiguous segmented tensor_reduce.
  - LSTM interleaved one step per group behind the conv pipeline; whh/x
    projections as DoubleRow (K=256 in one MM); sigmoid via tanh(x/2)
    (keeps ACT in the exp/tanh table set); h stored as H=2h in fp8 with
    weight scales folded on host.
  - sentence attention: same exp-direct softmax; sigmoid output via tanh.

Data-parallel over docs: core k handles docs 4k..4k+3; host concatenates
the 8 [1,4] outputs. No collectives.
"""
import sys

for _p in ("/opt/trn_rl_repo", "/root/.axon_site/_ro/trn_rl_repo"):
    if _p not in sys.path:
        sys.path.insert(0, _p)

from contextlib import ExitStack

import numpy as np
import ml_dtypes

import concourse.bass as bass
from concourse import mybir
from concourse.bass import IndirectOffsetOnAxis
from concourse.bass_utils import run_bass_kernel_spmd
from concourse.tile import TileContext

from concourse.vector_clock import ScopedClock


class _TC(TileContext):
    """TileContext that limits every instruction to a single sem wait
    (this walrus rejects multiple sync waits on one instruction); extra
    waits are spilled onto preceding same-engine nops."""

    def _commit_instruction(self, inst, lazy_reg_writes: bool = True):
        si = getattr(inst, "sync_info", None)
        if (
            si is not None
            and si.on_wait
            and len(si.on_wait) > 1
            and inst.engine != mybir.EngineType.Unassigned
        ):
            waits = list(si.on_wait)
            inst.sync_info = mybir.SyncInfo(
                on_wait=[waits[-1]], on_update=list(si.on_update or []))
            eng = self.nc.engines[inst.engine]
            for w in waits[:-1]:
                nop = eng.nop().ins
                nop.sync_info = mybir.SyncInfo(on_wait=[w], on_update=[])
        return super()._commit_instruction(inst, lazy_reg_writes)

    def _drain_and_barrier(self, tick_clock, wait_clock):
        carrier = self.nc.sync.nop().ins
        wait_clock.add_sem_waits(
            carrier, ScopedClock({None: tick_clock.global_clock}))
        si = carrier.sync_info
        if si is not None and si.on_wait and len(si.on_wait) > 1:
            waits = list(si.on_wait)
            carrier.sync_info = mybir.SyncInfo(
                on_wait=[waits[0]], on_update=list(si.on_update or []))
            for w in waits[1:]:
                n2 = self.nc.sync.nop().ins
                n2.sync_info = mybir.SyncInfo(on_wait=[w], on_update=[])
        self.nc.sync.drain()
        self.nc.all_engine_barrier()
        assert self.sems is not None
        popped = self.nc._tile_sem_poison_stack.pop()
        assert popped is self._sem_poison
        self.nc.clear_and_free_semaphores(list(self.sems.allocated().values()))
        self.nc.all_engine_barrier()


BF16 = mybir.dt.bfloat16
FP32 = mybir.dt.float32
FP8 = mybir.dt.float8e4
INT32 = mybir.dt.int32
AF = mybir.ActivationFunctionType
OP = mybir.AluOpType
AX = mybir.AxisListType
DR = mybir.MatmulPerfMode.DoubleRow

B, S, L = 32, 64, 128
PER = 32
TOTAL = B * PER
V, E, F, W, H = 30000, 300, 256, 5, 256
T = L - W + 1            # 124 valid conv positions
NCORES = 8
DPC = B // NCORES        # 4 docs per core
bf16 = ml_dtypes.bfloat16
f8 = ml_dtypes.float8_e4m3

LSTM_LAG = 5             # lstm step t issues during group t+LSTM_LAG
XP_CHUNK = 4             # x-projection batched every 4 slots


def build_nc(S_eff: int, debug_taps=False):
    assert S_eff % XP_CHUNK == 0
    NLOC = S_eff * DPC
    S4 = S_eff * DPC

    nc = bass.Bass()
    dbg = {}
    if debug_taps:
        dbg["c0"] = nc.dram_tensor("dbg_c0", [128, 2, 528], FP8,
                                   kind="ExternalOutput")
        dbg["s0"] = nc.dram_tensor("dbg_s0", [128, 2, S_eff, DPC], FP8,
                                   kind="ExternalOutput")
        dbg["xpt"] = nc.dram_tensor("dbg_xpt", [128, 8, S_eff, DPC], FP32,
                                    kind="ExternalOutput")
        dbg["hsb"] = nc.dram_tensor("dbg_hsb", [128, 2, S_eff, DPC], FP8,
                                    kind="ExternalOutput")
        dbg["ebt"] = nc.dram_tensor("dbg_ebt", [128, 2, 528], FP8,
                                    kind="ExternalOutput")
        dbg["tail"] = nc.dram_tensor("dbg_tail", [45, 3, 528], FP8,
                                     kind="ExternalOutput")

    # ---- DRAM I/O ----
    d_idx = nc.dram_tensor("idx_t", [128, NLOC], INT32, kind="ExternalInput")
    d_wemb = nc.dram_tensor("wemb", [V, E], FP32, kind="ExternalInput")
    d_wdr = nc.dram_tensor("wdr", [128, 2, W, 2, 128], FP8, kind="ExternalInput")
    d_wtail = nc.dram_tensor("wtail", [45, 2, 3, 2, 128], FP8,
                             kind="ExternalInput")
    d_wa0 = nc.dram_tensor("wa0dr", [128, 2, 2, 128], FP8, kind="ExternalInput")
    d_wih = nc.dram_tensor("wih", [128, 2, 8, 128], FP8, kind="ExternalInput")
    d_bx = nc.dram_tensor("bihhh_x", [128, 8, XP_CHUNK, DPC], FP32,
                          kind="ExternalInput")
    d_whh = nc.dram_tensor("whh", [128, 2, 8, 128], FP8, kind="ExternalInput")
    d_wa1 = nc.dram_tensor("wa1dr", [128, 2, 2, 128], FP8, kind="ExternalInput")
    d_wo = nc.dram_tensor("wo_t", [128, 2], BF16, kind="ExternalInput")
    d_boh = nc.dram_tensor("bo_half", [1, 1], FP32, kind="ExternalInput")
    d_out = nc.dram_tensor("out", [1, DPC], FP32, kind="ExternalOutput")

    with _TC(nc) as tc, ExitStack() as ctx:
        consts = ctx.enter_context(tc.tile_pool(name="consts", bufs=1))

        idx_sb = consts.tile([128, NLOC], INT32)
        nc.sync.dma_start(out=idx_sb[:, :], in_=d_idx[:, :])
        wdr_sb = consts.tile([128, 2, W, 2, 128], FP8)
        nc.sync.dma_start(out=wdr_sb[:, :, :, :, :], in_=d_wdr[:, :, :, :, :])
        wtail_sb = consts.tile([45, 2, 3, 2, 128], FP8)
        nc.sync.dma_start(out=wtail_sb[:, :, :, :, :],
                          in_=d_wtail[:, :, :, :, :])
        wa0_sb = consts.tile([128, 2, 2, 128], FP8)
        nc.sync.dma_start(out=wa0_sb[:, :, :, :], in_=d_wa0[:, :, :, :])
        wih_sb = consts.tile([128, 2, 8, 128], FP8)
        nc.sync.dma_start(out=wih_sb[:, :, :, :], in_=d_wih[:, :, :, :])
        bx_sb = consts.tile([128, 8, XP_CHUNK, DPC], FP32)
        nc.sync.dma_start(out=bx_sb[:, :, :, :], in_=d_bx[:, :, :, :])
        whh_sb = consts.tile([128, 2, 8, 128], FP8)
        nc.sync.dma_start(out=whh_sb[:, :, :, :], in_=d_whh[:, :, :, :])
        wa1_sb = consts.tile([128, 2, 2, 128], FP8)
        nc.sync.dma_start(out=wa1_sb[:, :, :, :], in_=d_wa1[:, :, :, :])
        wo_sb = consts.tile([128, 2], BF16)
        nc.sync.dma_start(out=wo_sb[:, :], in_=d_wo[:, :])
        boh_sb = consts.tile([1, 1], FP32)
        nc.sync.dma_start(out=boh_sb[:, :], in_=d_boh[:, :])

        # persistent state
        s0T = consts.tile([128, 2, S_eff, DPC], FP8)
        xpt = consts.tile([128, 8, S_eff, DPC], FP32)
        hsb = consts.tile([128, 2, S_eff, DPC], FP8)
        cC = consts.tile([128, 2, DPC], FP32)          # C = 2*c state
        nc.vector.memset(cC[:, :, :], 0.0)

        ident32 = consts.tile([128, 128], FP32)
        from concourse.masks import make_identity
        make_identity(nc, ident32[:, :])
        identf8 = consts.tile([128, 128], FP8)
        nc.vector.tensor_copy(out=identf8[:, :], in_=ident32[:, :])
        # fp8 gathered-embedding staging (manually rotated, 3 deep)
        emb8_0 = consts.tile([128, DPC, E], FP8)
        emb8_1 = consts.tile([128, DPC, E], FP8)
        emb8_2 = consts.tile([128, DPC, E], FP8)
        emb8s = [emb8_0, emb8_1, emb8_2]
        # tail fp8: slot 0 = ebT[c-4], slot 1 = ebT[c-3] (+ row 44 ones for
        # the conv bias), slot 2 = zeros (DR filler half for the w4 tap)
        tail_0 = consts.tile([45, 3, 528], FP8)
        tail_1 = consts.tile([45, 3, 528], FP8)
        tail_2 = consts.tile([45, 3, 528], FP8)
        tails = [tail_0, tail_1, tail_2]
        for tl in tails:
            nc.vector.memset(tl[0:45, 2, :], 0.0)
            nc.vector.memset(tl[32:45, 1, :], 1.0)

        with (
            tc.tile_pool(name="emb", bufs=5) as p_emb,
            tc.tile_pool(name="ebt8", bufs=3) as p_ebt8,
            tc.tile_pool(name="c0", bufs=2) as p_c0,
            tc.tile_pool(name="ex", bufs=2) as p_ex,
            tc.tile_pool(name="small", bufs=3) as p_small,
            tc.tile_pool(name="lst", bufs=3) as p_lst,
            tc.tile_pool(name="mm", bufs=2, space="PSUM") as p_mm,
            tc.tile_pool(name="tg", bufs=2, space="PSUM") as p_tg,
            tc.tile_pool(name="tp", bufs=1, space="PSUM") as p_tp,
        ):
            embs = {}
            tps = {}
            ebt8s = {}

            def gather(g):
                emb = p_emb.tile([128, DPC, E], FP32, tag="emb")
                for d in range(DPC):
                    nc.gpsimd.indirect_dma_start(
                        out=emb[:, d, :],
                        out_offset=None,
                        in_=d_wemb[:, :],
                        in_offset=IndirectOffsetOnAxis(
                            ap=idx_sb[:, DPC * g + d:DPC * g + d + 1], axis=0),
                    )
                embs[g] = emb

            def cast1(g):
                # fp32 -> fp8 on the ACT engine
                nc.scalar.activation(out=emb8s[g % 3][:, :, :],
                                     in_=embs.pop(g)[:, :, :], func=AF.Copy)

            def transp(g):
                # PE transpose (fp8 matmul vs identity) into PSUM
                src = emb8s[g % 3]
                tpm = p_tp.tile([128, 2, 512], FP32, tag="tpm")
                tpt = p_tg.tile([128, 512], FP32, tag="tg")
                for s in range(DPC):
                    for ec in range(2):
                        nc.tensor.matmul(
                            out=tpm[:, ec, 128 * s:128 * (s + 1)],
                            lhsT=src[:, s, 128 * ec:128 * (ec + 1)],
                            rhs=identf8[:, :], start=True, stop=True)
                    nc.tensor.matmul(
                        out=tpt[0:44, 128 * s:128 * (s + 1)],
                        lhsT=src[:, s, 256:300],
                        rhs=identf8[:, :], start=True, stop=True)
                tps[g] = (tpm, tpt)

            def cast2_fixed(g):
                # PSUM fp32 -> fp8 staging for the DoubleRow conv
                tpm, tpt = tps.pop(g)
                ebt = p_ebt8.tile([128, 2, 528], FP8, tag="ebt")
                nc.vector.tensor_copy(out=ebt[:, :, 0:512], in_=tpm[:, :, :])
                tail = tails[g % 3]
                nc.vector.tensor_copy(out=tail[0:44, 0, 4:516],
                                      in_=tpt[0:44, 0:512])
                nc.vector.tensor_copy(out=tail[0:44, 1, 4:515],
                                      in_=tpt[0:44, 1:512])
                ebt8s[g] = (ebt, tail)

            pending_bias = []

            def xp_mms(j0):
                xps = p_tg.tile([128, 512], FP32, tag="tg")
                xv = xps[:, :].rearrange("p (g x) -> p g x", g=8)
                for gt in range(8):
                    nc.tensor.matmul(
                        out=xv[:, gt, 0:XP_CHUNK * DPC],
                        lhsT=wih_sb[:, 0:2, gt, :],
                        rhs=s0T[:, 0:2, j0:j0 + XP_CHUNK, :],
                        start=True, stop=True, perf_mode=DR,
                    )
                pending_bias.append((j0, xps))

            def xp_bias_flush():
                while pending_bias:
                    j0, xps = pending_bias.pop()
                    xv = xps[:, :].rearrange("p (g x) -> p g x", g=8)
                    nc.vector.tensor_tensor(
                        out=xpt[:, :, j0:j0 + XP_CHUNK, :],
                        in0=xv[:, :, 0:XP_CHUNK * DPC].rearrange(
                            "p g (j d) -> p g j d", d=DPC),
                        in1=bx_sb[:, :, :, :], op=OP.add)

            def lstm_step(t):
                if t == 0:
                    ga_ap = xpt[:, :, 0, :]
                else:
                    gps = p_tg.tile([128, 512], FP32, tag="tg")
                    gv = gps[:, :].rearrange("p (g x) -> p g x", g=8)
                    for gt in range(8):
                        nc.tensor.matmul(
                            out=gv[:, gt, 0:DPC],
                            lhsT=whh_sb[:, 0:2, gt, :],
                            rhs=hsb[:, 0:2, t - 1, :],
                            start=True, stop=True, perf_mode=DR,
                        )
                    ga = p_lst.tile([128, 8, DPC], FP32, tag="ga")
                    nc.vector.tensor_tensor(
                        out=ga[:, :, :], in0=gv[:, :, 0:DPC],
                        in1=xpt[:, :, t, :], op=OP.add)
                    ga_ap = ga[:, :, :]
                tga = p_lst.tile([128, 8, DPC], FP32, tag="tga")
                nc.scalar.activation(out=tga[:, :, :], in_=ga_ap,
                                     func=AF.Tanh, scale=0.5)
                a = p_lst.tile([128, 2, DPC], FP32, tag="a")
                b = p_lst.tile([128, 2, DPC], FP32, tag="b")
                # a = (t_f + 1) * C = 4 f c ; b = (t_i + 1) * g~ = 2 i g~
                nc.vector.scalar_tensor_tensor(
                    out=a[:, :, :], in0=tga[:, 2:4, :], scalar=1.0,
                    in1=cC[:, :, :], op0=OP.add, op1=OP.mult)
                nc.vector.scalar_tensor_tensor(
                    out=b[:, :, :], in0=tga[:, 0:2, :], scalar=1.0,
                    in1=tga[:, 4:6, :], op0=OP.add, op1=OP.mult)
                # C = a/2 + b  (= 2 c_new)
                nc.vector.scalar_tensor_tensor(
                    out=cC[:, :, :], in0=a[:, :, :], scalar=0.5,
                    in1=b[:, :, :], op0=OP.mult, op1=OP.add)
                tch = p_lst.tile([128, 2, DPC], FP32, tag="tch")
                nc.scalar.activation(out=tch[:, :, :], in_=cC[:, :, :],
                                     func=AF.Tanh, scale=0.5)
                # H = (t_o + 1) * tanh(c) = 2 h   (fp8)
                nc.vector.scalar_tensor_tensor(
                    out=hsb[:, :, t, :], in0=tga[:, 6:8, :], scalar=1.0,
                    in1=tch[:, :, :], op0=OP.add, op1=OP.mult)

            # ---- warmup ----
            for i in range(min(4, S_eff)):
                gather(i)
            for i in range(min(3, S_eff)):
                cast1(i)
            transp(0)
            cast2_fixed(0)
            if S_eff > 1:
                transp(1)

            for g in range(S_eff):
                if g + 4 < S_eff:
                    gather(g + 4)
                if g + 3 < S_eff:
                    cast1(g + 3)
                if g + 1 < S_eff:
                    cast2_fixed(g + 1)
                xp_bias_flush()
                if g - LSTM_LAG >= 0:
                    lstm_step(g - LSTM_LAG)

                ebt, tail = ebt8s.pop(g)

                # ---- conv: fp8 DoubleRow accumulation ----
                cps = p_mm.tile([128, 2, 512], FP32, tag="mm")
                for fc in range(2):
                    for w in range(W):
                        nc.tensor.matmul(
                            out=cps[:, fc, 0:508],
                            lhsT=wdr_sb[:, :, w, fc, :],
                            rhs=ebt[:, 0:2, w:w + 508],
                            start=(w == 0), stop=False, perf_mode=DR)
                    nc.tensor.matmul(
                        out=cps[:, fc, 0:508],
                        lhsT=wtail_sb[0:44, :, 0, fc, :],
                        rhs=tail[0:44, 0:2, 4:512],
                        start=False, stop=False, perf_mode=DR)
                    nc.tensor.matmul(
                        out=cps[:, fc, 0:508],
                        lhsT=wtail_sb[0:44, :, 1, fc, :],
                        rhs=tail[0:44, 0:2, 6:514],
                        start=False, stop=False, perf_mode=DR)
                    nc.tensor.matmul(
                        out=cps[:, fc, 0:508],
                        lhsT=wtail_sb[0:45, :, 2, fc, :],
                        rhs=tail[0:45, 1:3, 7:515],
                        start=False, stop=True, perf_mode=DR)

                # transposes for group g+2 (PE), behind the conv in the queue
                if g + 2 < S_eff:
                    transp(g + 2)

                c0 = p_c0.tile([128, 2, 528], FP8, tag="c0")
                nc.scalar.activation(out=c0[:, :, 0:512], in_=cps[:, :, :],
                                     func=AF.Tanh)
                if debug_taps and g == 0:
                    nc.sync.dma_start(out=dbg["c0"][:, :, :], in_=c0[:, :, :])
                    nc.sync.dma_start(out=dbg["ebt"][:, :, :], in_=ebt[:, :, :])
                    nc.sync.dma_start(out=dbg["tail"][:, :, :],
                                      in_=tail[:, :, :])

                # ---- token attention ----
                lps = p_mm.tile([128, 2, 512], FP32, tag="mm")
                for mc in range(2):
                    nc.tensor.matmul(
                        out=lps[:, mc, 0:508],
                        lhsT=wa0_sb[:, :, mc, :],
                        rhs=c0[:, 0:2, 0:508],
                        start=True, stop=True, perf_mode=DR)
                ex = p_ex.tile([128, 2, 512], BF16, tag="ex")
                nc.scalar.activation(out=ex[:, :, 0:512], in_=lps[:, :, :],
                                     func=AF.Exp)
                # zero the 4 pad columns after each sentence so the segmented
                # reductions below can run over contiguous 128-wide blocks
                nc.vector.memset(
                    ex[:, :, :].rearrange(
                        "p m (s t) -> p m s t", t=128)[:, :, :, T:128], 0.0)
                prod = p_ex.tile([128, 2, 512], BF16, tag="prod")
                nc.vector.tensor_tensor(
                    out=prod[:, :, :], in0=ex[:, :, :],
                    in1=c0[:, :, 0:512], op=OP.mult)
                num = p_small.tile([128, 2, DPC], BF16, tag="num")
                den = p_small.tile([128, 2, DPC], BF16, tag="den")
                with nc.allow_low_precision("softmax sums of ~124 bounded "
                                            "terms; fp8 path dominates error"):
                    nc.vector.tensor_reduce(
                        out=num[:, :, :],
                        in_=prod[:, :, :].rearrange(
                            "p m (s t) -> p m s t", t=128),
                        axis=AX.X, op=OP.add)
                    nc.vector.tensor_reduce(
                        out=den[:, :, :],
                        in_=ex[:, :, :].rearrange(
                            "p m (s t) -> p m s t", t=128),
                        axis=AX.X, op=OP.add)
                rden = p_small.tile([128, 2, DPC], FP32, tag="rden")
                nc.vector.reciprocal(out=rden[:, :, :], in_=den[:, :, :])
                nc.vector.tensor_tensor(
                    out=s0T[:, :, g, :], in0=num[:, :, :], in1=rden[:, :, :],
                    op=OP.mult)

                # ---- x-projection for the previous XP_CHUNK slots ----
                if g % XP_CHUNK == 0 and g >= XP_CHUNK:
                    xp_mms(g - XP_CHUNK)

            # ---- drain: last xp chunk + remaining LSTM steps ----
            xp_mms(S_eff - XP_CHUNK)
            xp_bias_flush()
            for t in range(max(0, S_eff - LSTM_LAG), S_eff):
                lstm_step(t)

            if debug_taps:
                nc.sync.dma_start(out=dbg["s0"][:, :, :, :],
                                  in_=s0T[:, :, :, :])
                nc.sync.dma_start(out=dbg["xpt"][:, :, :, :],
                                  in_=xpt[:, :, :, :])
                nc.sync.dma_start(out=dbg["hsb"][:, :, :, :],
                                  in_=hsb[:, :, :, :])

            # ---- sentence attention + output ----
            l1ps = p_mm.tile([128, 2, 512], FP32, tag="mm")
            for mc in range(2):
                nc.tensor.matmul(
                    out=l1ps[:, mc, 0:S4],
                    lhsT=wa1_sb[:, :, mc, :],
                    rhs=hsb[:, 0:2, :, :],
                    start=True, stop=True, perf_mode=DR)
            ex1 = p_ex.tile([128, 2, S4], BF16, tag="ex1")
            nc.scalar.activation(out=ex1[:, :, :], in_=l1ps[:, :, 0:S4],
                                 func=AF.Exp)
            prod1 = p_ex.tile([128, 2, S_eff, DPC], BF16, tag="prod1")
            nc.vector.tensor_tensor(
                out=prod1[:, :, :, :],
                in0=ex1[:, :, :].rearrange("p m (t d) -> p m t d", d=DPC),
                in1=hsb[:, :, :, :], op=OP.mult)
            num1 = p_small.tile([128, 2, DPC], FP32, tag="num1")
            den1 = p_small.tile([128, 2, DPC], FP32, tag="den1")
            nc.vector.tensor_reduce(
                out=num1[:, :, :],
                in_=prod1[:, :, :, :].rearrange("p m t d -> p m d t"),
                axis=AX.X, op=OP.add)
            nc.vector.tensor_reduce(
                out=den1[:, :, :],
                in_=ex1[:, :, :].rearrange(
                    "p m (t d) -> p m d t", d=DPC),
                axis=AX.X, op=OP.add)
            nc.vector.reciprocal(out=den1[:, :, :], in_=den1[:, :, :])
            s1 = p_small.tile([128, 2, DPC], BF16, tag="s1")
            nc.vector.tensor_tensor(
                out=s1[:, :, :], in0=num1[:, :, :], in1=den1[:, :, :],
                op=OP.mult)
            ops = p_tg.tile([128, 512], FP32, tag="tg")
            for kc in range(2):
                nc.tensor.matmul(
                    out=ops[0:1, 0:DPC],
                    lhsT=wo_sb[:, kc:kc + 1],
                    rhs=s1[:, kc, :],
                    start=(kc == 0), stop=(kc == 1),
                )
            y = p_small.tile([1, DPC], FP32, tag="y")
            nc.scalar.activation(
                out=y[:, :], in_=ops[0:1, 0:DPC],
                func=AF.Tanh, bias=boh_sb[0:1, 0:1], scale=0.5)
            nc.vector.tensor_scalar(
                out=y[:, :], in0=y[:, :],
                scalar1=0.5, scalar2=0.5, op0=OP.mult, op1=OP.add)
            nc.sync.dma_start(out=d_out[:, :], in_=y[:, :])

    return nc


def _host_prep(inputs):
    inp = {k: np.asarray(v) for k, v in inputs.items()}
    tok = inp["input"].astype(np.int32)
    num_sent = inp["num_sent"].astype(np.int64)

    assert np.all(num_sent == num_sent[0]), "non-uniform num_sent unsupported"
    S_eff = int(num_sent[0])
    assert S_eff % XP_CHUNK == 0 and S_eff >= XP_CHUNK

    wc = np.asarray(inp["Wconv"], np.float32)     # [F, 1, W, E]
    bconv = np.asarray(inp["bconv"], np.float32)  # [F]

    wdr = np.zeros((128, 2, W, 2, 128), f8)
    for w in range(W):
        for fc in range(2):
            for k2 in range(2):
                blk = wc[128 * fc:128 * (fc + 1), 0, w,
                         128 * k2:128 * (k2 + 1)]      # [m, p]
                wdr[:, k2, w, fc, :] = blk.T.astype(f8)
    wt = np.zeros((45, 2, 3, 2, 128), np.float32)      # [p,k2,pair,fc,m]
    for fc in range(2):
        msl = slice(128 * fc, 128 * (fc + 1))
        wt[0:44, 0, 0, fc, :] = wc[msl, 0, 0, 256:300].T   # w0
        wt[0:44, 1, 0, fc, :] = wc[msl, 0, 1, 256:300].T   # w1
        wt[0:44, 0, 1, fc, :] = wc[msl, 0, 2, 256:300].T   # w2
        wt[0:44, 1, 1, fc, :] = wc[msl, 0, 3, 256:300].T   # w3
        wt[0:44, 0, 2, fc, :] = wc[msl, 0, 4, 256:300].T   # w4
        wt[44, 0, 2, fc, :] = bconv[msl]                   # bias via ones row

    wa0 = np.asarray(inp["Wa0"], np.float32)
    wa0dr = np.zeros((128, 2, 2, 128), f8)
    for k2 in range(2):
        for mc in range(2):
            wa0dr[:, k2, mc, :] = wa0[128 * k2:128 * (k2 + 1),
                                      128 * mc:128 * (mc + 1)].astype(f8)

    # gate order (i0,i1,f0,f1,g0,g1,o0,o1); g-gate rows x2 (tanh via scale .5)
    wih = np.asarray(inp["Wih"], np.float32)      # [4H, F]
    whh = np.asarray(inp["Whh"], np.float32)      # [4H, H]
    bih = np.asarray(inp["bih"], np.float32) + np.asarray(inp["bhh"], np.float32)
    gscale = np.ones((4 * H, 1), np.float32)
    gscale[2 * H:3 * H] = 2.0
    wih_eff = wih * gscale
    whh_eff = (whh * 0.5) * gscale                 # h stored as H=2h
    bx1 = (bih * gscale[:, 0])                     # [4H]
    wih_t = np.zeros((128, 2, 8, 128), f8)
    whh_t = np.zeros((128, 2, 8, 128), f8)
    for kc in range(2):
        for gt in range(8):
            wih_t[:, kc, gt, :] = wih_eff[128 * gt:128 * (gt + 1),
                                          128 * kc:128 * (kc + 1)].T.astype(f8)
            whh_t[:, kc, gt, :] = whh_eff[128 * gt:128 * (gt + 1),
                                          128 * kc:128 * (kc + 1)].T.astype(f8)
    bx = np.zeros((128, 8, XP_CHUNK, DPC), np.float32)
    for gt in range(8):
        bx[:, gt, :, :] = bx1[128 * gt:128 * (gt + 1)][:, None, None]

    wa1 = np.asarray(inp["Wa1"], np.float32) * 0.5  # h = H/2
    wa1dr = np.zeros((128, 2, 2, 128), f8)
    for k2 in range(2):
        for mc in range(2):
            wa1dr[:, k2, mc, :] = wa1[128 * k2:128 * (k2 + 1),
                                      128 * mc:128 * (mc + 1)].astype(f8)

    wo = np.asarray(inp["Wo"], np.float32) * 0.5    # s1 arrives as 2*s1
    wo_t = wo[:, 0].reshape(2, 128).T.astype(bf16).copy()
    boh = (0.5 * np.asarray(inp["bo"], np.float32)).reshape(1, 1)

    wemb = np.ascontiguousarray(inp["Wemb"], np.float32)

    in_maps = []
    for k in range(NCORES):
        idx_t = np.zeros((128, S_eff * DPC), np.int32)
        for g in range(S_eff):
            for d in range(DPC):
                sent = (k * DPC + d) * PER + g
                idx_t[:, DPC * g + d] = tok[sent]
        in_maps.append({
            "idx_t": idx_t, "wemb": wemb,
            "wdr": np.ascontiguousarray(wdr),
            "wtail": np.ascontiguousarray(wt.astype(f8)),
            "wa0dr": wa0dr, "wih": wih_t, "bihhh_x": bx,
            "whh": whh_t, "wa1dr": wa1dr, "wo_t": wo_t, "bo_half": boh,
        })
    return S_eff, in_maps


_NC_CACHE = {}


def kernel(**inputs) -> np.ndarray:
    S_eff, in_maps = _host_prep(inputs)
    if S_eff not in _NC_CACHE:
        _NC_CACHE[S_eff] = build_nc(S_eff)
    nc = _NC_CACHE[S_eff]
    res = run_bass_kernel_spmd(nc, in_maps, core_ids=list(range(NCORES)))
    out = np.zeros((B, 1), np.float32)
    for k in range(NCORES):
        out[k * DPC:(k + 1) * DPC, 0] = res.results[k]["out"][0]
    return out
